# revision 2
# baseline (speedup 1.0000x reference)
"""Trainium2 Bass kernel for the batched multi-period portfolio QP
(projected subgradient descent, 200 iterations).

Strategy (per spec sharding hint): B=128 QP instances sharded 16 per core
across 8 NeuronCores; each core solves its 16*12 = 192 independent
128-dim QPs entirely on-chip.

Per-core algorithm:
  - Precompute Sigma2G[v] = 2*GAMMA * L_v @ L_v^T on the TensorEngine
    (fp32), stored in SBUF as fp16 (host-validated: final rel err ~5e-4).
  - 200 iterations of:
      W_T   = transpose(W)                     (PE, fp32)
      q_T   = Sigma2G[v] @ W_T[:, v]  (192 weight-path matmuls -> PSUM cols)
      s_T   = sign(W_T - shift_h(W_T))         (ACT Sign; shifts are free-dim
                                                AP offsets in transposed layout)
      grad_T = q_T - mu_T + COST*(s_T - s_next_T)
      grad  = transpose(grad_T)                (PE)
      v     = W - eta_k * grad                 (fused DVE op, eta from table)
      W     = proj_simplex(v) via 3 warm-started Newton rounds
              (theta state carried across iterations; fused relu+sum on ACT,
               fused mask+count on DVE)

The simplex projection here is Newton on phi(t) = sum(relu(v - t)) - 1,
which is exactly Michelot's method; warm-started it converges in <= 3
rounds on this problem (validated host-side end to end: rel err 2.5e-5
fp32 / 4.9e-4 with fp16 matvec vs the sort-based reference).
"""
import os

import numpy as np

import concourse.bass as bass
import concourse.mybir as mybir
import concourse.tile as tile
from concourse.bass_utils import run_bass_kernel_spmd
from concourse.vector_clock import ScopedClock

# ---------------------------------------------------------------------------
# Workaround for this container's walrus build, which only accepts a single
# sync-wait per instruction. Two pieces:
#   1. TileContext tail drain: spread its aggregated waits across extra
#      single-wait Drain instructions (sem-ge waits commute).
#   2. General post-pass: hoist excess waits from any instruction onto
#      injected single-wait NoOps on the same engine immediately before it
#      (per-engine program order preserved -> semantics preserved).
# ---------------------------------------------------------------------------


def _patched_drain_and_barrier(self, tick_clock, wait_clock):
    drain_inst = self.nc.sync.drain()
    wait_clock.add_sem_waits(
        drain_inst.ins, ScopedClock({None: tick_clock.global_clock})
    )
    si = drain_inst.ins.sync_info
    waits = list(si.on_wait or []) if si is not None else []
    if len(waits) > 1:
        drain_inst.ins.sync_info = mybir.SyncInfo(
            on_wait=[waits[0]], on_update=list(si.on_update or [])
        )
        for w in waits[1:]:
            extra = self.nc.sync.drain()
            extra.ins.sync_info = mybir.SyncInfo(on_wait=[w], on_update=[])
    self.nc.all_engine_barrier()
    assert self.sems is not None
    popped = self.nc._tile_sem_poison_stack.pop()
    assert popped is self._sem_poison
    self.nc.clear_and_free_semaphores(list(self.sems.allocated().values()))
    self.nc.all_engine_barrier()


tile.TileContext._drain_and_barrier = _patched_drain_and_barrier


def _legalize_sync_waits(nc, max_waits=1):
    n_split = 0
    for f in nc.m.functions:
        for b in f.blocks:
            il = b.instructions
            i = 0
            while i < len(il):
                inst = il[i]
                si = inst.sync_info
                if si is None:
                    i += 1
                    continue
                waits = list(si.on_wait or [])
                if len(waits) > max_waits:
                    keep = waits[:max_waits]
                    excess = waits[max_waits:]
                    inst.sync_info = mybir.SyncInfo(
                        on_wait=keep, on_update=list(si.on_update or [])
                    )
                    for w in excess:
                        nop = mybir.InstNoOp(
                            name=nc.get_next_instruction_name(),
                            engine=inst.engine,
                            ins=[],
                            outs=[],
                            sync_info=mybir.SyncInfo(on_wait=[w], on_update=[]),
                        )
                        nc.register_instruction(nop)
                        il.insert(i, nop)
                        i += 1
                        n_split += 1
                i += 1
    return n_split


# ---------------------------------------------------------------------------
# Problem constants (hardcoded per the task contract).
# ---------------------------------------------------------------------------
GAMMA = 5.0
COST = 1e-3
ITERS = int(os.environ.get("BASS_MPO_ITERS", "200"))
ETA0 = 0.02
NEWTON_ROUNDS = 3

N_CORES = 8
B, H, N = 128, 12, 128
BC = B // N_CORES          # batches per core
V = BC * H                 # QP instances per core (= 192)

F32 = mybir.dt.float32
F16 = mybir.dt.float16
AF = mybir.ActivationFunctionType
OP = mybir.AluOpType


def _build_nc():
    nc = bass.Bass("TRN2", target_bir_lowering=False, debug=False)

    Lw = nc.dram_tensor("Lw", [V * N, N], F32, kind="ExternalInput")
    NMU_T = nc.dram_tensor("NMU_T", [N, V], F32, kind="ExternalInput")
    WPREV_T = nc.dram_tensor("WPREV_T", [N, BC], F32, kind="ExternalInput")
    NEGETA = nc.dram_tensor("NEGETA", [N, max(ITERS, 1)], F32, kind="ExternalInput")
    IDT = nc.dram_tensor("IDT", [N, N], F32, kind="ExternalInput")
    WOUT = nc.dram_tensor("WOUT", [V, N], F32, kind="ExternalOutput")

    with tile.TileContext(nc) as tc:
        with tc.tile_pool(name="pers", bufs=1) as pers:
            idt = pers.tile([N, N], F32, tag="idt")
            nc.sync.dma_start(idt[:], IDT.ap())
            nmu = pers.tile([N, V], F32, tag="nmu")
            nc.sync.dma_start(nmu[:], NMU_T.ap())
            wprev = pers.tile([N, BC], F32, tag="wprev")
            nc.sync.dma_start(wprev[:], WPREV_T.ap())
            negeta = pers.tile([N, max(ITERS, 1)], F32, tag="negeta")
            nc.sync.dma_start(negeta[:], NEGETA.ap())

            sig16 = pers.tile([N, V * N], F16, tag="sig16")

            w0 = pers.tile([128, N], F32, tag="w0")
            nc.gpsimd.memset(w0[:], 1.0 / N)
            w1 = pers.tile([64, N], F32, tag="w1")
            nc.gpsimd.memset(w1[:], 1.0 / N)
            nth0 = pers.tile([128, 1], F32, tag="nth0")
            nc.gpsimd.memset(nth0[:], 0.0)
            nth1 = pers.tile([64, 1], F32, tag="nth1")
            nc.gpsimd.memset(nth1[:], 0.0)

            wt_sb = pers.tile([N, V], F32, tag="wt_sb")
            wt16 = pers.tile([N, V], F16, tag="wt16")
            dT = pers.tile([N, V], F32, tag="dT")
            sT = pers.tile([N, V], F32, tag="sT")
            tT = pers.tile([N, V], F32, tag="tT")

            # ---------------- Sigma precompute ----------------
            with tc.tile_pool(name="pre_ps", bufs=1, space="PSUM") as pps, \
                 tc.tile_pool(name="lstage", bufs=4) as lsp, \
                 tc.tile_pool(name="ltsb", bufs=3) as ltp:
                for v in range(V):
                    lst = lsp.tile([N, N], F32, tag="lst")
                    nc.sync.dma_start(lst[:], Lw.ap()[v * N:(v + 1) * N, :])
                    lt_ps = pps.tile([N, N], F32, tag="lt", bufs=2)
                    nc.tensor.transpose(lt_ps[:], lst[:], idt[:])
                    lt_sb = ltp.tile([N, N], F32, tag="ltsb")
                    nc.vector.tensor_copy(lt_sb[:], lt_ps[:])
                    sig_ps = pps.tile([N, N], F32, tag="sig", bufs=2)
                    nc.tensor.matmul(
                        sig_ps[:], lt_sb[:], lt_sb[:], start=True, stop=True
                    )
                    nc.scalar.mul(
                        sig16[:, v * N:(v + 1) * N], sig_ps[:], 2.0 * GAMMA
                    )

            # ---------------- iteration loop ----------------
            with tc.tile_pool(name="lps", bufs=1, space="PSUM") as lps, \
                 tc.tile_pool(name="scr", bufs=1) as scr:
                with tc.For_i(0, ITERS, 1) as k:
                    # W_T (fp32 for sign terms, fp16 for PE rhs)
                    wt0_ps = lps.tile([128, 128], F32, tag="wt0")
                    nc.tensor.transpose(wt0_ps[:], w0[:], idt[:])
                    wt1_ps = lps.tile([128, 64], F32, tag="wt1")
                    nc.tensor.transpose(wt1_ps[:], w1[:], idt[0:64, 0:64])
                    nc.vector.tensor_copy(wt_sb[:, 0:128], wt0_ps[:])
                    nc.vector.tensor_copy(wt_sb[:, 128:192], wt1_ps[:])
                    nc.scalar.copy(wt16[:], wt_sb[:])

                    # trade-diff sign terms (all shifts are free-dim offsets)
                    nc.vector.tensor_sub(dT[:, 0:BC], wt_sb[:, 0:BC], wprev[:])
                    nc.vector.tensor_sub(
                        dT[:, BC:V], wt_sb[:, BC:V], wt_sb[:, 0:V - BC]
                    )
                    nc.scalar.sign(sT[:], dT[:])
                    nc.vector.tensor_sub(
                        tT[:, 0:V - BC], sT[:, 0:V - BC], sT[:, BC:V]
                    )
                    nc.vector.tensor_copy(tT[:, V - BC:V], sT[:, V - BC:V])

                    # 192 matvecs: gwt[:, v] = Sigma2G_v @ w_v
                    gwt = lps.tile([N, V], F32, tag="gwt")
                    for v in range(V):
                        nc.tensor.matmul(
                            gwt[:, v:v + 1],
                            sig16[:, v * N:(v + 1) * N],
                            wt16[:, v:v + 1],
                            start=True,
                            stop=True,
                        )

                    # grad_T = (gwt - mu_T) + COST * t_T   (2 fused DVE ops)
                    f1 = scr.tile([N, V], F32, tag="f1")
                    nc.vector.tensor_add(f1[:], gwt[:], nmu[:])
                    gwt_sb = scr.tile([N, V], F32, tag="gwt_sb")
                    nc.vector.scalar_tensor_tensor(
                        gwt_sb[:], tT[:], COST, f1[:], op0=OP.mult, op1=OP.add
                    )

                    # transpose grad back to standard layout
                    gr0 = lps.tile([128, 128], F32, tag="gr0")
                    nc.tensor.transpose(gr0[:], gwt_sb[:, 0:128], idt[:])
                    gr1 = lps.tile([64, 128], F32, tag="gr1")
                    nc.tensor.transpose(gr1[:], gwt_sb[:, 128:192], idt[:])

                    # update + projection per partition tile
                    for wt, nth, gr, pn in (
                        (w0, nth0, gr0, 128),
                        (w1, nth1, gr1, 64),
                    ):
                        vv = scr.tile([pn, N], F32, tag=f"v{pn}")
                        nc.vector.scalar_tensor_tensor(
                            vv[:],
                            gr[:],
                            negeta[0:pn, bass.ds(k, 1)],
                            wt[:],
                            op0=OP.mult,
                            op1=OP.add,
                        )
                        for _ in range(NEWTON_ROUNDS):
                            rel = scr.tile([pn, N], F32, tag=f"rel{pn}")
                            sumr = scr.tile([pn, 1], F32, tag=f"sumr{pn}")
                            nc.scalar.activation(
                                rel[:], vv[:], AF.Relu,
                                bias=nth[:], scale=1.0, accum_out=sumr[:],
                            )
                            th = scr.tile([pn, 1], F32, tag=f"th{pn}")
                            nc.vector.tensor_scalar_mul(th[:], nth[:], -1.0)
                            # out = (v > theta); accum op1=add -> count
                            msk = scr.tile([pn, N], F32, tag=f"msk{pn}")
                            cnt = scr.tile([pn, 1], F32, tag=f"cnt{pn}")
                            nc.vector.tensor_scalar(
                                msk[:], vv[:], th[:], None,
                                op0=OP.is_gt, op1=OP.add, accum_out=cnt[:],
                            )
                            nc.vector.tensor_scalar_max(cnt[:], cnt[:], 1.0)
                            inv = scr.tile([pn, 1], F32, tag=f"inv{pn}")
                            nc.vector.reciprocal(inv[:], cnt[:])
                            dlt = scr.tile([pn, 1], F32, tag=f"dlt{pn}")
                            nc.vector.tensor_scalar(
                                dlt[:], sumr[:], -1.0, inv[:],
                                op0=OP.add, op1=OP.mult,
                            )
                            nc.vector.tensor_scalar_sub(nth[:], nth[:], dlt[:])
                        nc.scalar.activation(
                            wt[:], vv[:], AF.Relu, bias=nth[:], scale=1.0
                        )

                nc.sync.dma_start(WOUT.ap()[0:128, :], w0[:])
                nc.sync.dma_start(WOUT.ap()[128:192, :], w1[:])

    _legalize_sync_waits(nc)
    return nc


def kernel(mu, L, w_prev):
    mu = np.ascontiguousarray(np.asarray(mu, dtype=np.float32))
    L = np.ascontiguousarray(np.asarray(L, dtype=np.float32))
    w_prev = np.ascontiguousarray(np.asarray(w_prev, dtype=np.float32))

    eta = (ETA0 / np.sqrt(np.arange(1, ITERS + 1, dtype=np.float32))).astype(
        np.float32
    )
    negeta = np.ascontiguousarray(
        np.broadcast_to(-eta[None, :], (N, max(ITERS, 1))).astype(np.float32)
    )
    idt = np.eye(N, dtype=np.float32)

    in_maps = []
    for c in range(N_CORES):
        bs = slice(c * BC, (c + 1) * BC)
        # h-major instance order: v = h*BC + b_local
        Lw_c = np.ascontiguousarray(
            L[bs].transpose(1, 0, 2, 3).reshape(V * N, N)
        )
        nmu_c = np.ascontiguousarray(
            (-mu[bs]).transpose(2, 1, 0).reshape(N, V)
        )
        wprev_c = np.ascontiguousarray(w_prev[bs].T)
        in_maps.append(
            {
                "Lw": Lw_c,
                "NMU_T": nmu_c,
                "WPREV_T": wprev_c,
                "NEGETA": negeta,
                "IDT": idt,
            }
        )

    nc = _build_nc()
    res = run_bass_kernel_spmd(nc, in_maps, core_ids=list(range(N_CORES)))

    out = np.empty((B, H, N), dtype=np.float32)
    for c in range(N_CORES):
        wout = res.results[c]["WOUT"]  # [V, N], v = h*BC + b_local
        out[c * BC:(c + 1) * BC] = wout.reshape(H, BC, N).transpose(1, 0, 2)
    return out


# revision 8
# speedup vs baseline: 1.1011x; 1.1011x over previous
"""Trainium2 Bass kernel for the batched multi-period portfolio QP
(projected subgradient descent, 200 iterations).

Strategy (per spec sharding hint): B=128 QP instances sharded 16 per core
across 8 NeuronCores; each core solves its 16*12 = 192 independent
128-dim QPs entirely on-chip.

Per-core algorithm:
  - Precompute Sigma2G[v] = 2*GAMMA * L_v @ L_v^T on the TensorEngine
    (fp32), stored in SBUF as fp16 (host-validated: final rel err ~5e-4).
  - 200 iterations of:
      W_T   = transpose(W)                     (PE, fp32)
      q_T   = Sigma2G[v] @ W_T[:, v]  (192 weight-path matmuls -> PSUM cols)
      s_T   = sign(W_T - shift_h(W_T))         (ACT Sign; shifts are free-dim
                                                AP offsets in transposed layout)
      grad_T = q_T - mu_T + COST*(s_T - s_next_T)
      grad  = transpose(grad_T)                (PE)
      v     = W - eta_k * grad                 (fused DVE op, eta from table)
      W     = proj_simplex(v) via 3 warm-started Newton rounds
              (theta state carried across iterations; fused relu+sum on ACT,
               fused mask+count on DVE)

The simplex projection here is Newton on phi(t) = sum(relu(v - t)) - 1,
which is exactly Michelot's method; warm-started it converges in <= 3
rounds on this problem (validated host-side end to end: rel err 2.5e-5
fp32 / 4.9e-4 with fp16 matvec vs the sort-based reference).
"""
import os

import numpy as np

import concourse.bass as bass
import concourse.mybir as mybir
import concourse.tile as tile
from concourse.bass_utils import run_bass_kernel_spmd
from concourse.vector_clock import ScopedClock

# ---------------------------------------------------------------------------
# Workaround for this container's walrus build, which only accepts a single
# sync-wait per instruction. Two pieces:
#   1. TileContext tail drain: spread its aggregated waits across extra
#      single-wait Drain instructions (sem-ge waits commute).
#   2. General post-pass: hoist excess waits from any instruction onto
#      injected single-wait NoOps on the same engine immediately before it
#      (per-engine program order preserved -> semantics preserved).
# ---------------------------------------------------------------------------


def _patched_drain_and_barrier(self, tick_clock, wait_clock):
    drain_inst = self.nc.sync.drain()
    wait_clock.add_sem_waits(
        drain_inst.ins, ScopedClock({None: tick_clock.global_clock})
    )
    si = drain_inst.ins.sync_info
    waits = list(si.on_wait or []) if si is not None else []
    if len(waits) > 1:
        drain_inst.ins.sync_info = mybir.SyncInfo(
            on_wait=[waits[0]], on_update=list(si.on_update or [])
        )
        for w in waits[1:]:
            extra = self.nc.sync.drain()
            extra.ins.sync_info = mybir.SyncInfo(on_wait=[w], on_update=[])
    self.nc.all_engine_barrier()
    assert self.sems is not None
    popped = self.nc._tile_sem_poison_stack.pop()
    assert popped is self._sem_poison
    self.nc.clear_and_free_semaphores(list(self.sems.allocated().values()))
    self.nc.all_engine_barrier()


tile.TileContext._drain_and_barrier = _patched_drain_and_barrier


def _legalize_sync_waits(nc, max_waits=1):
    n_split = 0
    for f in nc.m.functions:
        for b in f.blocks:
            il = b.instructions
            i = 0
            while i < len(il):
                inst = il[i]
                si = inst.sync_info
                if si is None:
                    i += 1
                    continue
                waits = list(si.on_wait or [])
                if len(waits) > max_waits:
                    keep = waits[:max_waits]
                    excess = waits[max_waits:]
                    inst.sync_info = mybir.SyncInfo(
                        on_wait=keep, on_update=list(si.on_update or [])
                    )
                    for w in excess:
                        nop = mybir.InstNoOp(
                            name=nc.get_next_instruction_name(),
                            engine=inst.engine,
                            ins=[],
                            outs=[],
                            sync_info=mybir.SyncInfo(on_wait=[w], on_update=[]),
                        )
                        nc.register_instruction(nop)
                        il.insert(i, nop)
                        i += 1
                        n_split += 1
                i += 1
    return n_split


# ---------------------------------------------------------------------------
# Problem constants (hardcoded per the task contract).
# ---------------------------------------------------------------------------
GAMMA = 5.0
COST = 1e-3
ITERS = int(os.environ.get("BASS_MPO_ITERS", "200"))
ETA0 = 0.02
NEWTON_ROUNDS = 3
# temporary bisection knobs (timing experiments only; all default off)
_SKIP_MATVECS = os.environ.get("BASS_MPO_SKIP_MATVECS", "0") == "1"
_SKIP_GRADPROJ = os.environ.get("BASS_MPO_SKIP_GRADPROJ", "0") == "1"
_HINT_PE = os.environ.get("BASS_MPO_HINT_PE", "0") == "1"
_STAGGER = os.environ.get("BASS_MPO_STAGGER", "0") == "1"

N_CORES = 8
B, H, N = 128, 12, 128
BC = B // N_CORES          # batches per core
V = BC * H                 # QP instances per core (= 192)

F32 = mybir.dt.float32
F16 = mybir.dt.float16
AF = mybir.ActivationFunctionType
OP = mybir.AluOpType


def _build_nc():
    nc = bass.Bass("TRN2", target_bir_lowering=False, debug=False)

    Lw = nc.dram_tensor("Lw", [V * N, N], F32, kind="ExternalInput")
    NMU_T = nc.dram_tensor("NMU_T", [N, V], F32, kind="ExternalInput")
    WPREV_T = nc.dram_tensor("WPREV_T", [N, BC], F32, kind="ExternalInput")
    NEGETA = nc.dram_tensor("NEGETA", [N, max(ITERS, 1)], F32, kind="ExternalInput")
    IDT = nc.dram_tensor("IDT", [N, N], F32, kind="ExternalInput")
    WOUT = nc.dram_tensor("WOUT", [V, N], F32, kind="ExternalOutput")

    with tile.TileContext(nc) as tc:
        with tc.tile_pool(name="pers", bufs=1) as pers:
            idt = pers.tile([N, N], F32, tag="idt")
            nc.sync.dma_start(idt[:], IDT.ap())
            nmu = pers.tile([N, V], F32, tag="nmu")
            nc.sync.dma_start(nmu[:], NMU_T.ap())
            wprev = pers.tile([N, BC], F32, tag="wprev")
            nc.sync.dma_start(wprev[:], WPREV_T.ap())
            negeta = pers.tile([N, max(ITERS, 1)], F32, tag="negeta")
            nc.sync.dma_start(negeta[:], NEGETA.ap())

            sig16 = pers.tile([N, V * N], F16, tag="sig16")

            w0 = pers.tile([128, N], F32, tag="w0")
            nc.gpsimd.memset(w0[:], 1.0 / N)
            w1 = pers.tile([64, N], F32, tag="w1")
            nc.gpsimd.memset(w1[:], 1.0 / N)
            nth0 = pers.tile([128, 1], F32, tag="nth0")
            nc.gpsimd.memset(nth0[:], 0.0)
            nth1 = pers.tile([64, 1], F32, tag="nth1")
            nc.gpsimd.memset(nth1[:], 0.0)

            wt_sb = pers.tile([N, V], F32, tag="wt_sb")
            wt16 = pers.tile([N, V], F16, tag="wt16")
            dT = pers.tile([N, V], F32, tag="dT")
            sT = pers.tile([N, V], F32, tag="sT")
            tT = pers.tile([N, V], F32, tag="tT")

            # ---------------- Sigma precompute ----------------
            with tc.tile_pool(name="pre_ps", bufs=1, space="PSUM") as pps, \
                 tc.tile_pool(name="lstage", bufs=4) as lsp, \
                 tc.tile_pool(name="ltsb", bufs=3) as ltp:
                for v in range(V):
                    lst = lsp.tile([N, N], F32, tag="lst")
                    nc.sync.dma_start(lst[:], Lw.ap()[v * N:(v + 1) * N, :])
                    lt_ps = pps.tile([N, N], F32, tag="lt", bufs=2)
                    nc.tensor.transpose(lt_ps[:], lst[:], idt[:])
                    lt_sb = ltp.tile([N, N], F32, tag="ltsb")
                    nc.vector.tensor_copy(lt_sb[:], lt_ps[:])
                    sig_ps = pps.tile([N, N], F32, tag="sig", bufs=2)
                    nc.tensor.matmul(
                        sig_ps[:], lt_sb[:], lt_sb[:], start=True, stop=True
                    )
                    nc.scalar.mul(
                        sig16[:, v * N:(v + 1) * N], sig_ps[:], 2.0 * GAMMA
                    )

            # ---------------- iteration loop ----------------
            with tc.tile_pool(name="lps", bufs=1, space="PSUM") as lps, \
                 tc.tile_pool(name="scr", bufs=1) as scr:
                _loop_kw = {}
                if _HINT_PE:
                    _loop_kw["hint_engines"] = (mybir.EngineType.PE,)
                if _STAGGER:
                    _loop_kw["staggered_reset"] = True
                _outer = int(os.environ.get("BASS_MPO_OUTER", "1"))
                if _outer > 1:
                    _octx = tc.For_i(0, _outer, 1)
                    _octx.__enter__()
                with tc.For_i(0, ITERS, 1, **_loop_kw) as k:
                    # W_T (fp32 for sign terms, fp16 for PE rhs)
                    wt0_ps = lps.tile([128, 128], F32, tag="wt0")
                    nc.tensor.transpose(wt0_ps[:], w0[:], idt[:])
                    wt1_ps = lps.tile([128, 64], F32, tag="wt1")
                    nc.tensor.transpose(wt1_ps[:], w1[:], idt[0:64, 0:64])
                    nc.vector.tensor_copy(wt_sb[:, 0:128], wt0_ps[:])
                    nc.vector.tensor_copy(wt_sb[:, 128:192], wt1_ps[:])
                    nc.scalar.copy(wt16[:], wt_sb[:])

                    # trade-diff sign terms (all shifts are free-dim offsets)
                    nc.vector.tensor_sub(dT[:, 0:BC], wt_sb[:, 0:BC], wprev[:])
                    nc.vector.tensor_sub(
                        dT[:, BC:V], wt_sb[:, BC:V], wt_sb[:, 0:V - BC]
                    )
                    nc.scalar.sign(sT[:], dT[:])
                    nc.vector.tensor_sub(
                        tT[:, 0:V - BC], sT[:, 0:V - BC], sT[:, BC:V]
                    )
                    nc.vector.tensor_copy(tT[:, V - BC:V], sT[:, V - BC:V])

                    # 192 matvecs: gwt[:, v] = Sigma2G_v @ w_v
                    gwt = lps.tile([N, V], F32, tag="gwt")
                    for v in range(0 if _SKIP_MATVECS else V):
                        nc.tensor.matmul(
                            gwt[:, v:v + 1],
                            sig16[:, v * N:(v + 1) * N],
                            wt16[:, v:v + 1],
                            start=True,
                            stop=True,
                        )

                    def _grad_proj():
                        # grad_T = (gwt - mu_T) + COST * t_T  (2 fused DVE ops)
                        f1 = scr.tile([N, V], F32, tag="f1")
                        nc.vector.tensor_add(f1[:], gwt[:], nmu[:])
                        gwt_sb = scr.tile([N, V], F32, tag="gwt_sb")
                        nc.vector.scalar_tensor_tensor(
                            gwt_sb[:], tT[:], COST, f1[:], op0=OP.mult, op1=OP.add
                        )

                        # transpose grad back to standard layout
                        gr0 = lps.tile([128, 128], F32, tag="gr0")
                        nc.tensor.transpose(gr0[:], gwt_sb[:, 0:128], idt[:])
                        gr1 = lps.tile([64, 128], F32, tag="gr1")
                        nc.tensor.transpose(gr1[:], gwt_sb[:, 128:192], idt[:])

                        # update + projection per partition tile
                        for wt, nth, gr, pn in (
                            (w0, nth0, gr0, 128),
                            (w1, nth1, gr1, 64),
                        ):
                            vv = scr.tile([pn, N], F32, tag=f"v{pn}")
                            nc.vector.scalar_tensor_tensor(
                                vv[:],
                                gr[:],
                                negeta[0:pn, bass.ds(k, 1)],
                                wt[:],
                                op0=OP.mult,
                                op1=OP.add,
                            )
                            for _ in range(NEWTON_ROUNDS):
                                rel = scr.tile([pn, N], F32, tag=f"rel{pn}")
                                sumr = scr.tile([pn, 1], F32, tag=f"sumr{pn}")
                                nc.scalar.activation(
                                    rel[:], vv[:], AF.Relu,
                                    bias=nth[:], scale=1.0, accum_out=sumr[:],
                                )
                                th = scr.tile([pn, 1], F32, tag=f"th{pn}")
                                nc.vector.tensor_scalar_mul(th[:], nth[:], -1.0)
                                # out = (v > theta); accum op1=add -> count
                                msk = scr.tile([pn, N], F32, tag=f"msk{pn}")
                                cnt = scr.tile([pn, 1], F32, tag=f"cnt{pn}")
                                nc.vector.tensor_scalar(
                                    msk[:], vv[:], th[:], None,
                                    op0=OP.is_gt, op1=OP.add, accum_out=cnt[:],
                                )
                                nc.vector.tensor_scalar_max(cnt[:], cnt[:], 1.0)
                                inv = scr.tile([pn, 1], F32, tag=f"inv{pn}")
                                nc.vector.reciprocal(inv[:], cnt[:])
                                dlt = scr.tile([pn, 1], F32, tag=f"dlt{pn}")
                                nc.vector.tensor_scalar(
                                    dlt[:], sumr[:], -1.0, inv[:],
                                    op0=OP.add, op1=OP.mult,
                                )
                                nc.vector.tensor_scalar_sub(
                                    nth[:], nth[:], dlt[:]
                                )
                            nc.scalar.activation(
                                wt[:], vv[:], AF.Relu, bias=nth[:], scale=1.0
                            )

                    if not _SKIP_GRADPROJ:
                        _grad_proj()

                if _outer > 1:
                    _octx.__exit__(None, None, None)
                nc.sync.dma_start(WOUT.ap()[0:128, :], w0[:])
                nc.sync.dma_start(WOUT.ap()[128:192, :], w1[:])

    _legalize_sync_waits(nc)
    return nc


def kernel(mu, L, w_prev):
    mu = np.ascontiguousarray(np.asarray(mu, dtype=np.float32))
    L = np.ascontiguousarray(np.asarray(L, dtype=np.float32))
    w_prev = np.ascontiguousarray(np.asarray(w_prev, dtype=np.float32))

    eta = (ETA0 / np.sqrt(np.arange(1, ITERS + 1, dtype=np.float32))).astype(
        np.float32
    )
    negeta = np.ascontiguousarray(
        np.broadcast_to(-eta[None, :], (N, max(ITERS, 1))).astype(np.float32)
    )
    idt = np.eye(N, dtype=np.float32)

    in_maps = []
    for c in range(N_CORES):
        bs = slice(c * BC, (c + 1) * BC)
        # h-major instance order: v = h*BC + b_local
        Lw_c = np.ascontiguousarray(
            L[bs].transpose(1, 0, 2, 3).reshape(V * N, N)
        )
        nmu_c = np.ascontiguousarray(
            (-mu[bs]).transpose(2, 1, 0).reshape(N, V)
        )
        wprev_c = np.ascontiguousarray(w_prev[bs].T)
        in_maps.append(
            {
                "Lw": Lw_c,
                "NMU_T": nmu_c,
                "WPREV_T": wprev_c,
                "NEGETA": negeta,
                "IDT": idt,
            }
        )

    nc = _build_nc()
    res = run_bass_kernel_spmd(nc, in_maps, core_ids=list(range(N_CORES)))

    out = np.empty((B, H, N), dtype=np.float32)
    for c in range(N_CORES):
        wout = res.results[c]["WOUT"]  # [V, N], v = h*BC + b_local
        out[c * BC:(c + 1) * BC] = wout.reshape(H, BC, N).transpose(1, 0, 2)
    return out
